# revision 29
# baseline (speedup 1.0000x reference)
"""Trainium2 Bass kernel for nn_LlamaForSequenceRegression_14336600834254.

2-layer Llama (D=2048, H=16, HD=128, F=5632, LoRA r=16 on q/v) + regression
head, B=2, S=1024, fp32 reference.

Distribution (8 NeuronCores): DP2 x TP4.
  - cores 0-3 process batch 0, cores 4-7 batch 1 (data parallel).
  - within each group of 4: Megatron tensor parallel — Wq/Wk/Wv column
    shards (4 heads/core), Wo row shards, Wgate/Wup column shards
    (F/4=1408), Wdown row shards. AllReduce (bf16) after attn-out and
    after MLP-down, replica_groups=[[0,1,2,3],[4,5,6,7]].
  - embedding gather + norm-weight folding are done host-side; all
    device matmuls run in bf16 with fp32 PSUM accumulation; the
    residual stream / softmax / rmsnorm statistics are fp32.

Layout: activations are kept feature-major ("transposed"): h^T [D, T] as
SBUF tiles [128 part, 16 kchunk, 1024 tok] so every weight matmul uses the
natural [in, out] weight layout as lhsT and no transposes are needed.
Attention uses scores^T [Tk, Tq] so softmax needs no max-subtraction
(|scores| < ~6 with folded 1/sqrt(HD)) and probs feed the v-matmul
directly; the causal mask is an upload-once 0/1 strip multiplied into the
diagonal tiles, and the attention_mask rides the exp() per-partition bias.
"""

import numpy as np
import ml_dtypes

import concourse.bacc as bacc
import concourse.tile as tile
from concourse import mybir
from concourse import bass_utils

BF16 = ml_dtypes.bfloat16
FP32 = np.float32

V, D, L, H, HD, F, R, ALPHA, B, S, OUT = 32000, 2048, 2, 16, 128, 5632, 16, 32, 2, 1024, 11
EPS = 1e-5
SCALING = ALPHA / R
N_CORES = 8
TP = 4
NH = H // TP          # 4 local heads
DL = NH * HD          # 512 local q/k/v cols
FL = F // TP          # 1408 local mlp cols
KC = D // 128         # 16 contraction chunks
FC = FL // 128        # 11 mlp chunks
TT = 512              # token tile (free dim per matmul)
NT = S // TT          # 2 token tiles
TC = S // 128         # 8 token chunks (128-wide)
REPLICA_GROUPS = [[0, 1, 2, 3], [4, 5, 6, 7]]

dt = mybir.dt


def build_program():
    """Build the SPMD Bass program (identical on all 8 cores; weights differ
    per core via the input maps)."""
    nc = bacc.Bacc(num_devices=N_CORES, debug=False)

    # ---- DRAM I/O ----
    xT = nc.dram_tensor("xT", [128, KC, S], dt.bfloat16, kind="ExternalInput")
    cosT = nc.dram_tensor("cosT", [128, S], dt.bfloat16, kind="ExternalInput")
    sinT = nc.dram_tensor("sinT", [128, S], dt.bfloat16, kind="ExternalInput")
    mstrip = nc.dram_tensor("mstrip", [128, 896], dt.bfloat16, kind="ExternalInput")
    maskbias = nc.dram_tensor("maskbias", [128, TC], dt.float32, kind="ExternalInput")
    wreg = nc.dram_tensor("wreg", [KC, 128, OUT], dt.bfloat16, kind="ExternalInput")
    breg = nc.dram_tensor("breg", [OUT, 1], dt.float32, kind="ExternalInput")
    W = {}
    for l in range(L):
        W[f"wq{l}"] = nc.dram_tensor(f"wq{l}", [KC, 128, DL], dt.bfloat16, kind="ExternalInput")
        W[f"wk{l}"] = nc.dram_tensor(f"wk{l}", [KC, 128, DL], dt.bfloat16, kind="ExternalInput")
        W[f"wv{l}"] = nc.dram_tensor(f"wv{l}", [KC, 128, DL], dt.bfloat16, kind="ExternalInput")
        W[f"aq{l}"] = nc.dram_tensor(f"aq{l}", [KC, 128, R], dt.bfloat16, kind="ExternalInput")
        W[f"av{l}"] = nc.dram_tensor(f"av{l}", [KC, 128, R], dt.bfloat16, kind="ExternalInput")
        W[f"bq{l}"] = nc.dram_tensor(f"bq{l}", [R, DL], dt.bfloat16, kind="ExternalInput")
        W[f"bv{l}"] = nc.dram_tensor(f"bv{l}", [R, DL], dt.bfloat16, kind="ExternalInput")
        W[f"wo{l}"] = nc.dram_tensor(f"wo{l}", [NH, 128, D], dt.bfloat16, kind="ExternalInput")
        W[f"wg{l}"] = nc.dram_tensor(f"wg{l}", [FC, KC, 128, 128], dt.bfloat16, kind="ExternalInput")
        W[f"wu{l}"] = nc.dram_tensor(f"wu{l}", [FC, KC, 128, 128], dt.bfloat16, kind="ExternalInput")
        W[f"wd{l}"] = nc.dram_tensor(f"wd{l}", [FC, 128, D], dt.bfloat16, kind="ExternalInput")
    out_dram = nc.dram_tensor("out", [OUT, 1], dt.float32, kind="ExternalOutput")

    with tile.TileContext(nc) as tc:
        with (
            tc.tile_pool(name="persist", bufs=1) as pp,
            tc.tile_pool(name="wts", bufs=3) as wp,
            tc.tile_pool(name="colw", bufs=1) as cwp,
            tc.tile_pool(name="tmp", bufs=3) as tp_,
            tc.tile_pool(name="stage", bufs=2) as stp,
            tc.tile_pool(name="psum", bufs=8, space="PSUM") as ps,
            tc.tile_pool(name="dram", bufs=1, space="DRAM") as dram,
        ):
            f32, bf = dt.float32, dt.bfloat16
            # ---- persistent tiles ----
            h = pp.tile([128, KC, S], f32, tag="h")
            hn = pp.tile([128, KC, S], bf, tag="hn")
            cos_sb = pp.tile([128, S], bf, tag="cos")
            sin_sb = pp.tile([128, S], bf, tag="sin")
            mstrip_sb = pp.tile([128, 896], bf, tag="mstrip")
            mb_sb = pp.tile([128, TC], f32, tag="mb")
            ones_sb = pp.tile([128, 1], f32, tag="ones")
            oneD_sb = pp.tile([128, 1], f32, tag="oneD")
            ones_bf = pp.tile([128, 1], bf, tag="onesbf")
            eps_sb = pp.tile([1, 1], f32, tag="eps")
            ones_row = pp.tile([1, 128], f32, tag="ones_row")
            qT = pp.tile([128, NH, S], bf, tag="qT")
            kT = pp.tile([128, NH, S], bf, tag="kT")
            vN = pp.tile([128, TC, DL], bf, tag="vN")
            ctxT = pp.tile([128, NH, S], bf, tag="ctxT")
            expT = pp.tile([128, TC, TT], bf, tag="expT")
            mT = pp.tile([128, FC, S], bf, tag="mT")
            rs_bc = pp.tile([128, S], f32, tag="rs_bc")
            rden_bc = pp.tile([128, TT], f32, tag="rden_bc")
            rs1 = pp.tile([1, S], f32, tag="rs1")
            rden1 = pp.tile([1, S], f32, tag="rden1")
            aqw = pp.tile([128, KC, R], bf, tag="aqw")
            avw = pp.tile([128, KC, R], bf, tag="avw")
            bq_sb = pp.tile([R, DL], bf, tag="bq")
            bv_sb = pp.tile([R, DL], bf, tag="bv")
            aq_sb = pp.tile([R, S], bf, tag="aq")
            av_sb = pp.tile([R, S], bf, tag="av")
            wreg_sb = pp.tile([128, KC, OUT], bf, tag="wreg")
            breg_sb = pp.tile([OUT, 1], f32, tag="breg")

            # ---- constants in ----
            nc.vector.memset(ones_sb[:], 1.0)
            nc.vector.memset(oneD_sb[:], 1.0 / D)
            nc.vector.memset(ones_bf[:], 1.0)
            nc.vector.memset(eps_sb[:], EPS)
            nc.vector.memset(ones_row[:], 1.0)
            nc.sync.dma_start(cos_sb[:], cosT[:])
            nc.sync.dma_start(sin_sb[:], sinT[:])
            nc.sync.dma_start(mstrip_sb[:], mstrip[:])
            nc.sync.dma_start(mb_sb[:], maskbias[:])
            nc.sync.dma_start(breg_sb[:], breg[:])
            for k in range(KC):
                nc.sync.dma_start(wreg_sb[:, k, :], wreg[k])

            # ---- h init: bf16 upload -> fp32 residual ----
            nc.sync.dma_start(hn[:], xT[:])
            nc.vector.tensor_copy(h[:], hn[:])

            # DRAM bounce buffers for collectives
            ar_in = dram.tile([128, KC, S], bf)
            ar_out = dram.tile([128, KC, S], bf)
            ar1h_in = [dram.tile([128, KC, TT], bf, name=f"ar1hi_{t}") for t in range(NT)]
            ar1h_out = [dram.tile([128, KC, TT], bf, name=f"ar1ho_{t}") for t in range(NT)]
            ar2h_in = [dram.tile([128, KC, TT], bf, name=f"ar2hi_{t}") for t in range(NT)]
            ar2h_out = [dram.tile([128, KC, TT], bf, name=f"ar2ho_{t}") for t in range(NT)]

            def norm_to_hn():
                sums = []
                for t in range(NT):
                    sumsq = ps.tile([128, TT], f32, tag="psum")
                    sums.append(sumsq)
                for k in range(KC):
                    for t in range(NT):
                        sq = tp_.tile([128, TT], f32, tag="sq", bufs=2,
                                      name=f"sq_{k}_{t}")
                        nc.scalar.activation(sq[:], h[:, k, t * TT:(t + 1) * TT],
                                             mybir.ActivationFunctionType.Square)
                        nc.tensor.matmul(
                            sums[t][0:1, :], oneD_sb[:], sq[:],
                            start=(k == 0), stop=(k == KC - 1),
                        )
                for t in range(NT):
                    nc.scalar.activation(
                        rs1[0:1, t * TT:(t + 1) * TT], sums[t][0:1, :],
                        mybir.ActivationFunctionType.Sqrt, bias=eps_sb[:],
                    )
                nc.vector.reciprocal(rs1[0:1, :], rs1[0:1, :])
                nc.gpsimd.partition_broadcast(rs_bc[:], rs1[:])
                for k in range(KC):
                    nc.vector.tensor_mul(hn[:, k, :], h[:, k, :], rs_bc[:])

            def norm_to_hn_half(t):
                ts_ = slice(t * TT, (t + 1) * TT)
                sumsq = ps.tile([128, TT], f32, tag="psum", name=f"nsum_{t}")
                for k in range(KC):
                    sq = tp_.tile([128, TT], f32, tag="sq", bufs=2, name=f"nsq_{k}_{t}")
                    nc.scalar.activation(sq[:], h[:, k, ts_],
                                         mybir.ActivationFunctionType.Square)
                    nc.tensor.matmul(sumsq[0:1, :], oneD_sb[:], sq[:],
                                     start=(k == 0), stop=(k == KC - 1))
                nc.scalar.activation(rs1[0:1, ts_], sumsq[0:1, :],
                                     mybir.ActivationFunctionType.Sqrt, bias=eps_sb[:])
                nc.vector.reciprocal(rs1[0:1, ts_], rs1[0:1, ts_])
                # broadcast rs over partitions via PE outer product (keeps the
                # gpsimd queue free for collectives)
                psrs = ps.tile([128, TT], f32, tag="psum", name=f"psrs_{t}")
                nc.tensor.matmul(psrs[:], ones_row[:], rs1[0:1, ts_],
                                 start=True, stop=True)
                for k in range(KC):
                    nc.vector.tensor_mul(hn[:, k, ts_], h[:, k, ts_], psrs[:])

            def lora_down(aw, dst):
                """dst [R,S] bf16 = aw.T @ hn  (aw: [128,KC,R])."""
                for t in range(NT):
                    psa = ps.tile([128, TT], f32, tag="psum")
                    for k in range(KC):
                        nc.tensor.matmul(
                            psa[0:R, :], aw[:, k, :], hn[:, k, t * TT:(t + 1) * TT],
                            start=(k == 0), stop=(k == KC - 1),
                        )
                    nc.scalar.copy(dst[:, t * TT:(t + 1) * TT], psa[0:R, :])

            def rope_from_psum(psq, dst, hc, t):
                """Apply RoPE to psum [128,TT] (one head, token tile t) and
                write bf16 into dst[:, hc, t*TT:...]."""
                ts_ = slice(t * TT, (t + 1) * TT)
                # cos/sin tiles carry the same 64-row table duplicated into
                # both partition halves so every SB+SB operand pair below is
                # base-partition aligned.
                t2 = tp_.tile([128, TT], bf, tag="ropetB", bufs=1)
                t4 = tp_.tile([128, TT], bf, tag="ropetB", bufs=1)
                nc.vector.tensor_mul(dst[0:64, hc, ts_], psq[0:64, :], cos_sb[0:64, ts_])
                nc.vector.tensor_mul(t2[0:64, :], psq[64:128, :], sin_sb[0:64, ts_])
                nc.vector.tensor_sub(dst[0:64, hc, ts_], dst[0:64, hc, ts_], t2[0:64, :])
                nc.vector.tensor_mul(dst[64:128, hc, ts_], psq[64:128, :], cos_sb[64:128, ts_])
                nc.vector.tensor_mul(t4[64:128, :], psq[0:64, :], sin_sb[64:128, ts_])
                nc.vector.tensor_add(dst[64:128, hc, ts_], dst[64:128, hc, ts_], t4[64:128, :])

            def qk_proj(wname, dst, lora_bw, lora_act):
                """dst[:, hc, :] (bf16, roped) = rope(W.T @ hn [+ lora])."""
                psq = [[ps.tile([128, TT], f32, tag="psum", name=f"psq_{hc}_{t}")
                        for t in range(NT)] for hc in range(NH)]
                for k in range(KC):
                    wt = wp.tile([128, DL], bf, tag="wqkv", name=f"w_{wname}_{k}")
                    nc.sync.dma_start(wt[:], W[wname][k])
                    for hc in range(NH):
                        for t in range(NT):
                            nc.tensor.matmul(
                                psq[hc][t][:], wt[:, hc * HD:(hc + 1) * HD],
                                hn[:, k, t * TT:(t + 1) * TT],
                                start=(k == 0), stop=False,
                            )
                for hc in range(NH):
                    for t in range(NT):
                        if lora_bw is not None:
                            nc.tensor.matmul(
                                psq[hc][t][:], lora_bw[:, hc * HD:(hc + 1) * HD],
                                lora_act[:, t * TT:(t + 1) * TT],
                                start=False, stop=True,
                            )
                        else:
                            # close the accumulation group with a zero-op?
                            # instead mark the last k matmul as stop below.
                            pass
                        rope_from_psum(psq[hc][t], dst, hc, t)

            def qk_proj_nolora(wname, dst):
                psq = [[ps.tile([128, TT], f32, tag="psum", name=f"psqn_{hc}_{t}")
                        for t in range(NT)] for hc in range(NH)]
                for k in range(KC):
                    wt = wp.tile([128, DL], bf, tag="wqkv", name=f"wn_{wname}_{k}")
                    nc.sync.dma_start(wt[:], W[wname][k])
                    for hc in range(NH):
                        for t in range(NT):
                            nc.tensor.matmul(
                                psq[hc][t][:], wt[:, hc * HD:(hc + 1) * HD],
                                hn[:, k, t * TT:(t + 1) * TT],
                                start=(k == 0), stop=(k == KC - 1),
                            )
                for hc in range(NH):
                    for t in range(NT):
                        rope_from_psum(psq[hc][t], dst, hc, t)

            def v_proj(l):
                """vN [128(tok), TC, DL] bf16 = hn.T @ Wv + lora."""
                psv = [ps.tile([128, DL], f32, tag="psum", name=f"psv_{c}")
                       for c in range(TC)]
                for k in range(KC):
                    wt = wp.tile([128, DL], bf, tag="wqkv", name=f"wv_t_{k}")
                    nc.sync.dma_start(wt[:], W[f"wv{l}"][k])
                    for c in range(TC):
                        nc.tensor.matmul(
                            psv[c][:], hn[:, k, c * 128:(c + 1) * 128], wt[:],
                            start=(k == 0), stop=False,
                        )
                for c in range(TC):
                    nc.tensor.matmul(
                        psv[c][:], av_sb[:, c * 128:(c + 1) * 128], bv_sb[:],
                        start=False, stop=True,
                    )
                    nc.scalar.copy(vN[:, c, :], psv[c][:])

            def attention_half(t):
                """qT,kT,vN -> ctxT for token tile t (all local heads)."""
                if True:
                    for hh in range(NH):
                        ts_ = slice(t * TT, (t + 1) * TT)
                        jmax = (t + 1) * (TT // 128)
                        for j in range(jmax):
                            pss = ps.tile([128, TT], f32, tag="psum")
                            nc.tensor.matmul(
                                pss[:], kT[:, hh, j * 128:(j + 1) * 128],
                                qT[:, hh, ts_], start=True, stop=True,
                            )
                            nc.scalar.activation(
                                expT[:, j, :], pss[:],
                                mybir.ActivationFunctionType.Exp,
                                bias=mb_sb[:, j:j + 1], scale=1.0,
                            )
                            off = t * TT - j * 128
                            if off < 128:
                                # diagonal tile: multiply 0/1 causal strip
                                # allowed iff p <= f + off
                                nc.vector.tensor_mul(
                                    expT[:, j, :], expT[:, j, :],
                                    mstrip_sb[:, 384 + off:896 + off],
                                )
                        # denominator
                        psd = ps.tile([128, TT], f32, tag="psum")
                        for j in range(jmax):
                            nc.tensor.matmul(
                                psd[0:1, :], ones_bf[:], expT[:, j, :],
                                start=(j == 0), stop=(j == jmax - 1),
                            )
                        nc.vector.reciprocal(rden1[0:1, ts_], psd[0:1, :])
                        nc.gpsimd.partition_broadcast(rden_bc[:], rden1[:, ts_])
                        # ctx
                        psc = ps.tile([128, TT], f32, tag="psum")
                        for j in range(jmax):
                            nc.tensor.matmul(
                                psc[:], vN[:, j, hh * HD:(hh + 1) * HD],
                                expT[:, j, :],
                                start=(j == 0), stop=(j == jmax - 1),
                            )
                        nc.vector.tensor_mul(ctxT[:, hh, ts_], psc[:], rden_bc[:])

            def out_proj_half(l, t):
                """attn partial for token half t -> ar1h_in[t]."""
                for og in range(4):  # groups of 4 output chunks
                    pso = [ps.tile([128, TT], f32, tag="psum", name=f"pso_{og}_{oi}_{t}")
                           for oi in range(4)]
                    for hc in range(NH):
                        wt = wp.tile([128, TT], bf, tag="wqkv", name=f"wo_t_{og}_{hc}_{t}")
                        nc.sync.dma_start(
                            wt[:], W[f"wo{l}"][hc][:, og * 512:(og + 1) * 512])
                        for oi in range(4):
                            nc.tensor.matmul(
                                pso[oi][:], wt[:, oi * 128:(oi + 1) * 128],
                                ctxT[:, hc, t * TT:(t + 1) * TT],
                                start=(hc == 0), stop=(hc == NH - 1),
                            )
                    for oi in range(4):
                        st = stp.tile([128, TT], bf, tag="stage")
                        nc.scalar.copy(st[:], pso[oi][:])
                        nc.sync.dma_start(ar1h_in[t][:, og * 4 + oi, :], st[:])

            def allreduce1_half(t):
                nc.gpsimd.collective_compute(
                    "AllReduce", mybir.AluOpType.add,
                    replica_groups=REPLICA_GROUPS,
                    ins=[ar1h_in[t].opt()], outs=[ar1h_out[t].opt()],
                )

            def add1_half(t):
                ts_ = slice(t * TT, (t + 1) * TT)
                for k in range(KC):
                    nc.sync.dma_start(hn[:, k, ts_], ar1h_out[t][:, k, :])
                    nc.vector.tensor_add(h[:, k, ts_], h[:, k, ts_], hn[:, k, ts_])

            def allreduce2_half(t):
                nc.gpsimd.collective_compute(
                    "AllReduce", mybir.AluOpType.add,
                    replica_groups=REPLICA_GROUPS,
                    ins=[ar2h_in[t].opt()], outs=[ar2h_out[t].opt()],
                )

            def add2_half(t):
                ts_ = slice(t * TT, (t + 1) * TT)
                for k in range(KC):
                    nc.sync.dma_start(hn[:, k, ts_], ar2h_out[t][:, k, :])
                    nc.vector.tensor_add(h[:, k, ts_], h[:, k, ts_], hn[:, k, ts_])

            def lora_down_half(aw, dst, t):
                psa = ps.tile([128, TT], f32, tag="psum", name=f"psld_{t}")
                for k in range(KC):
                    nc.tensor.matmul(
                        psa[0:R, :], aw[:, k, :], hn[:, k, t * TT:(t + 1) * TT],
                        start=(k == 0), stop=(k == KC - 1),
                    )
                nc.scalar.copy(dst[:, t * TT:(t + 1) * TT], psa[0:R, :])

            def k_proj_half(wname, dst, t):
                psq = [ps.tile([128, TT], f32, tag="psum", name=f"psqh_{hc}_{t}")
                       for hc in range(NH)]
                for k in range(KC):
                    wt = wp.tile([128, DL], bf, tag="wqkv", name=f"wkh_{k}_{t}")
                    nc.sync.dma_start(wt[:], W[wname][k])
                    for hc in range(NH):
                        nc.tensor.matmul(
                            psq[hc][:], wt[:, hc * HD:(hc + 1) * HD],
                            hn[:, k, t * TT:(t + 1) * TT],
                            start=(k == 0), stop=(k == KC - 1),
                        )
                for hc in range(NH):
                    rope_from_psum(psq[hc], dst, hc, t)

            def v_proj_half(l, t):
                psv = [ps.tile([128, DL], f32, tag="psum", name=f"psvh_{c}_{t}")
                       for c in range(4)]
                for k in range(KC):
                    wt = wp.tile([128, DL], bf, tag="wqkv", name=f"wvh_{k}_{t}")
                    nc.sync.dma_start(wt[:], W[f"wv{l}"][k])
                    for ci in range(4):
                        c = t * 4 + ci
                        nc.tensor.matmul(
                            psv[ci][:], hn[:, k, c * 128:(c + 1) * 128], wt[:],
                            start=(k == 0), stop=False,
                        )
                for ci in range(4):
                    c = t * 4 + ci
                    nc.tensor.matmul(
                        psv[ci][:], av_sb[:, c * 128:(c + 1) * 128], bv_sb[:],
                        start=False, stop=True,
                    )
                    nc.scalar.copy(vN[:, c, :], psv[ci][:])

            def mlp_gate_up_half(l, t):
                ts_ = slice(t * TT, (t + 1) * TT)
                for fc in range(FC):
                    wg_t = cwp.tile([128, KC, 128], bf, tag="wgcol", name=f"wg_{fc}_{t}")
                    wu_t = cwp.tile([128, KC, 128], bf, tag="wucol", name=f"wu_{fc}_{t}")
                    nc.sync.dma_start(wg_t[:], W[f"wg{l}"][fc].rearrange("k p c -> p k c"))
                    nc.sync.dma_start(wu_t[:], W[f"wu{l}"][fc].rearrange("k p c -> p k c"))
                    psg = ps.tile([128, TT], f32, tag="psum", name=f"psg_{fc}_{t}")
                    psu = ps.tile([128, TT], f32, tag="psum", name=f"psu_{fc}_{t}")
                    for k in range(KC):
                        nc.tensor.matmul(psg[:], wg_t[:, k, :], hn[:, k, ts_],
                                         start=(k == 0), stop=(k == KC - 1))
                        nc.tensor.matmul(psu[:], wu_t[:, k, :], hn[:, k, ts_],
                                         start=(k == 0), stop=(k == KC - 1))
                    sg = tp_.tile([128, TT], bf, tag="silu", bufs=2, name=f"sg_{fc}_{t}")
                    nc.scalar.activation(sg[:], psg[:], mybir.ActivationFunctionType.Silu)
                    nc.vector.tensor_mul(mT[:, fc, ts_], sg[:], psu[:])

            def mlp_down_half(l, t):
                ts_ = slice(t * TT, (t + 1) * TT)
                for og in range(4):
                    pso = [ps.tile([128, TT], f32, tag="psum", name=f"psd_{og}_{oi}_{t}")
                           for oi in range(4)]
                    for kc in range(FC):
                        wt = wp.tile([128, TT], bf, tag="wqkv", name=f"wd_t_{og}_{kc}_{t}")
                        nc.sync.dma_start(
                            wt[:], W[f"wd{l}"][kc][:, og * 512:(og + 1) * 512])
                        for oi in range(4):
                            nc.tensor.matmul(
                                pso[oi][:], wt[:, oi * 128:(oi + 1) * 128],
                                mT[:, kc, ts_],
                                start=(kc == 0), stop=(kc == FC - 1),
                            )
                    for oi in range(4):
                        st = stp.tile([128, TT], bf, tag="stage")
                        nc.scalar.copy(st[:], pso[oi][:])
                        nc.sync.dma_start(ar2h_in[t][:, og * 4 + oi, :], st[:])

            def allreduce_and_add():
                nc.gpsimd.collective_compute(
                    "AllReduce", mybir.AluOpType.add,
                    replica_groups=REPLICA_GROUPS,
                    ins=[ar_in.opt()], outs=[ar_out.opt()],
                )
                # land the reduced partial in hn (dead here) and add to h
                for k in range(KC):
                    nc.sync.dma_start(hn[:, k, :], ar_out[:, k, :])
                    nc.vector.tensor_add(h[:, k, :], h[:, k, :], hn[:, k, :])


            # ---------- last-token ("tail") variants for the final layer:
            # everything after layer-1 K/V only influences the last token,
            # so q/attention/Wo/MLP run on one token and the AllReduces
            # shrink to [D,1].
            ar_in_s = dram.tile([128, KC, 1], bf)
            ar_out_s = dram.tile([128, KC, 1], bf)
            q_last = pp.tile([128, NH, 1], bf, tag="q_last")
            ctx_last = pp.tile([128, NH, 1], bf, tag="ctx_last")
            hn_last = pp.tile([128, KC, 1], bf, tag="hn_last")
            m_last = pp.tile([128, FC, 1], bf, tag="m_last")

            def q_proj_tail(l):
                psq = [ps.tile([128, TT], f32, tag="psum", name=f"psqt_{hc}")
                       for hc in range(NH)]
                for k in range(KC):
                    wt = wp.tile([128, DL], bf, tag="wqkv", name=f"wqt_{k}")
                    nc.sync.dma_start(wt[:], W[f"wq{l}"][k])
                    for hc in range(NH):
                        nc.tensor.matmul(
                            psq[hc][:, 0:1], wt[:, hc * HD:(hc + 1) * HD],
                            hn[:, k, S - 1:S], start=(k == 0), stop=False,
                        )
                for hc in range(NH):
                    nc.tensor.matmul(
                        psq[hc][:, 0:1], bq_sb[:, hc * HD:(hc + 1) * HD],
                        aq_sb[:, S - 1:S], start=False, stop=True,
                    )
                    # rope on one column
                    t2 = tp_.tile([128, 1], f32, tag="ropetS", bufs=2)
                    t4 = tp_.tile([128, 1], f32, tag="ropetS", bufs=2)
                    nc.vector.tensor_mul(q_last[0:64, hc, :], psq[hc][0:64, 0:1], cos_sb[0:64, S - 1:S])
                    nc.vector.tensor_mul(t2[0:64, :], psq[hc][64:128, 0:1], sin_sb[0:64, S - 1:S])
                    nc.vector.tensor_sub(q_last[0:64, hc, :], q_last[0:64, hc, :], t2[0:64, :])
                    nc.vector.tensor_mul(q_last[64:128, hc, :], psq[hc][64:128, 0:1], cos_sb[64:128, S - 1:S])
                    nc.vector.tensor_mul(t4[64:128, :], psq[hc][0:64, 0:1], sin_sb[64:128, S - 1:S])
                    nc.vector.tensor_add(q_last[64:128, hc, :], q_last[64:128, hc, :], t4[64:128, :])

            def attention_tail():
                # last token attends to every key: no causal strip needed.
                for hh in range(NH):
                    exps = tp_.tile([128, TC], bf, tag="expt_tail", bufs=2,
                                    name=f"expt_{hh}")
                    for j in range(TC):
                        pss = ps.tile([128, TT], f32, tag="psum", name=f"psst_{hh}_{j}")
                        nc.tensor.matmul(pss[:, 0:1], kT[:, hh, j * 128:(j + 1) * 128],
                                         q_last[:, hh, :], start=True, stop=True)
                        nc.scalar.activation(exps[:, j:j + 1], pss[:, 0:1],
                                             mybir.ActivationFunctionType.Exp,
                                             bias=mb_sb[:, j:j + 1], scale=1.0)
                    psd = ps.tile([128, TT], f32, tag="psum", name=f"psdt_{hh}")
                    psc = ps.tile([128, TT], f32, tag="psum", name=f"psct_{hh}")
                    for j in range(TC):
                        nc.tensor.matmul(psd[0:1, 0:1], ones_bf[:], exps[:, j:j + 1],
                                         start=(j == 0), stop=(j == TC - 1))
                        nc.tensor.matmul(psc[:, 0:1], vN[:, j, hh * HD:(hh + 1) * HD],
                                         exps[:, j:j + 1],
                                         start=(j == 0), stop=(j == TC - 1))
                    rd = tp_.tile([1, 1], f32, tag="rd_tail", bufs=2, name=f"rd_{hh}")
                    nc.vector.reciprocal(rd[:], psd[0:1, 0:1])
                    rdb = tp_.tile([128, 1], f32, tag="rdb_tail", bufs=2, name=f"rdb_{hh}")
                    nc.gpsimd.partition_broadcast(rdb[:], rd[:])
                    nc.vector.tensor_mul(ctx_last[:, hh, :], psc[:, 0:1], rdb[:])

            def out_proj_tail(l):
                for og in range(2):
                    pso = [ps.tile([128, TT], f32, tag="psum", name=f"psot_{og}_{oi}")
                           for oi in range(8)]
                    for hc in range(NH):
                        wta = wp.tile([128, TT], bf, tag="wqkv", name=f"wota_{og}_{hc}")
                        wtb = wp.tile([128, TT], bf, tag="wqkv", name=f"wotb_{og}_{hc}")
                        nc.sync.dma_start(
                            wta[:], W[f"wo{l}"][hc][:, og * 1024:og * 1024 + 512])
                        nc.sync.dma_start(
                            wtb[:], W[f"wo{l}"][hc][:, og * 1024 + 512:(og + 1) * 1024])
                        for oi in range(8):
                            wt_ = wta if oi < 4 else wtb
                            nc.tensor.matmul(
                                pso[oi][:, 0:1], wt_[:, (oi % 4) * 128:(oi % 4 + 1) * 128],
                                ctx_last[:, hc, :], start=(hc == 0), stop=(hc == NH - 1),
                            )
                    st = stp.tile([128, 8], bf, tag="stage_tail", bufs=2,
                                  name=f"stt_{og}")
                    for oi in range(8):
                        nc.scalar.copy(st[:, oi:oi + 1], pso[oi][:, 0:1])
                    nc.sync.dma_start(
                        ar_in_s[:, og * 8:(og + 1) * 8, 0], st[:])

            def allreduce_and_add_tail():
                nc.gpsimd.collective_compute(
                    "AllReduce", mybir.AluOpType.add,
                    replica_groups=REPLICA_GROUPS,
                    ins=[ar_in_s.opt()], outs=[ar_out_s.opt()],
                )
                lb = tp_.tile([128, KC, 1], bf, tag="ar_land", bufs=2)
                nc.sync.dma_start(lb[:], ar_out_s[:])
                nc.vector.tensor_add(h[:, :, S - 1:S], h[:, :, S - 1:S], lb[:])

            def norm_tail_to_hn_last():
                psl = ps.tile([128, TT], f32, tag="psum", name="psl_norm_tail")
                sqt = tp_.tile([128, KC, 1], f32, tag="sqlast")
                nc.scalar.activation(sqt[:], h[:, :, S - 1:S],
                                     mybir.ActivationFunctionType.Square)
                for k in range(KC):
                    nc.tensor.matmul(psl[0:1, 0:1], oneD_sb[:], sqt[:, k, :],
                                     start=(k == 0), stop=(k == KC - 1))
                rst = tp_.tile([1, 1], f32, tag="rst_tail")
                nc.scalar.activation(rst[:], psl[0:1, 0:1],
                                     mybir.ActivationFunctionType.Sqrt, bias=eps_sb[:])
                nc.vector.reciprocal(rst[:], rst[:])
                rstb = tp_.tile([128, 1], f32, tag="rstb_tail")
                nc.gpsimd.partition_broadcast(rstb[:], rst[:])
                nc.vector.tensor_mul(
                    hn_last[:], h[:, :, S - 1:S],
                    rstb[:, :, None].broadcast_to([128, 1, 1]).broadcast_to([128, KC, 1]),
                )

            def mlp_tail(l):
                norm_tail_to_hn_last()
                for fc in range(FC):
                    wg_t = cwp.tile([128, KC, 128], bf, tag="wgcol", name=f"wgt_{fc}")
                    wu_t = cwp.tile([128, KC, 128], bf, tag="wucol", name=f"wut_{fc}")
                    nc.sync.dma_start(wg_t[:], W[f"wg{l}"][fc].rearrange("k p c -> p k c"))
                    nc.sync.dma_start(wu_t[:], W[f"wu{l}"][fc].rearrange("k p c -> p k c"))
                    psg = ps.tile([128, TT], f32, tag="psum", name=f"psgt_{fc}")
                    psu = ps.tile([128, TT], f32, tag="psum", name=f"psut_{fc}")
                    for k in range(KC):
                        nc.tensor.matmul(psg[:, 0:1], wg_t[:, k, :], hn_last[:, k, :],
                                         start=(k == 0), stop=(k == KC - 1))
                        nc.tensor.matmul(psu[:, 0:1], wu_t[:, k, :], hn_last[:, k, :],
                                         start=(k == 0), stop=(k == KC - 1))
                    sgt = tp_.tile([128, 1], bf, tag="silut", bufs=2, name=f"sgt_{fc}")
                    nc.scalar.activation(sgt[:], psg[:, 0:1],
                                         mybir.ActivationFunctionType.Silu)
                    nc.vector.tensor_mul(m_last[:, fc, :], sgt[:], psu[:, 0:1])
                for og in range(2):
                    pso = [ps.tile([128, TT], f32, tag="psum", name=f"psdt2_{og}_{oi}")
                           for oi in range(8)]
                    for kc in range(FC):
                        wta = wp.tile([128, TT], bf, tag="wqkv", name=f"wdta_{og}_{kc}")
                        wtb = wp.tile([128, TT], bf, tag="wqkv", name=f"wdtb_{og}_{kc}")
                        nc.sync.dma_start(
                            wta[:], W[f"wd{l}"][kc][:, og * 1024:og * 1024 + 512])
                        nc.sync.dma_start(
                            wtb[:], W[f"wd{l}"][kc][:, og * 1024 + 512:(og + 1) * 1024])
                        for oi in range(8):
                            wt_ = wta if oi < 4 else wtb
                            nc.tensor.matmul(
                                pso[oi][:, 0:1], wt_[:, (oi % 4) * 128:(oi % 4 + 1) * 128],
                                m_last[:, kc, :], start=(kc == 0), stop=(kc == FC - 1),
                            )
                    st = stp.tile([128, 8], bf, tag="stage_tail", bufs=2,
                                  name=f"stt2_{og}")
                    for oi in range(8):
                        nc.scalar.copy(st[:, oi:oi + 1], pso[oi][:, 0:1])
                    nc.sync.dma_start(ar_in_s[:, og * 8:(og + 1) * 8, 0], st[:])

            # ================= layers =================
            for l in range(L):
                tail = (l == L - 1)
                for k in range(KC):
                    nc.sync.dma_start(aqw[:, k, :], W[f"aq{l}"][k])
                    nc.sync.dma_start(avw[:, k, :], W[f"av{l}"][k])
                nc.sync.dma_start(bq_sb[:], W[f"bq{l}"][:])
                nc.sync.dma_start(bv_sb[:], W[f"bv{l}"][:])
                if not tail:
                    norm_to_hn()
                    lora_down(aqw, aq_sb)
                    lora_down(avw, av_sb)
                    qk_proj_nolora(f"wk{l}", kT)
                    v_proj(l)
                else:
                    # previous layer's MLP AR lands per token half; k/v for
                    # this layer start on each half as soon as it arrives.
                    for t in range(NT):
                        add2_half(t)
                        norm_to_hn_half(t)
                        lora_down_half(avw, av_sb, t)
                        if t == NT - 1:
                            lora_down_half(aqw, aq_sb, t)
                        k_proj_half(f"wk{l}", kT, t)
                        v_proj_half(l, t)
                if not tail:
                    qk_proj(f"wq{l}", qT, bq_sb, aq_sb)
                    for t in range(NT):
                        attention_half(t)
                        out_proj_half(l, t)
                        allreduce1_half(t)
                    for t in range(NT):
                        add1_half(t)
                        norm_to_hn_half(t)
                        mlp_gate_up_half(l, t)
                        mlp_down_half(l, t)
                        allreduce2_half(t)
                else:
                    q_proj_tail(l)
                    attention_tail()
                    out_proj_tail(l)
                    allreduce_and_add_tail()
                    mlp_tail(l)
                    allreduce_and_add_tail()

            # ================= final norm + head (last token only) ========
            sq = tp_.tile([128, KC, 1], f32, tag="sqlast")
            nc.scalar.activation(sq[:], h[:, :, S - 1:S], mybir.ActivationFunctionType.Square)
            psl = ps.tile([128, TT], f32, tag="psum")
            for k in range(KC):
                nc.tensor.matmul(psl[0:1, 0:1], oneD_sb[:], sq[:, k, :],
                                 start=(k == 0), stop=(k == KC - 1))
            rsl = tp_.tile([1, 1], f32, tag="rsl")
            nc.scalar.activation(rsl[:], psl[0:1, 0:1],
                                 mybir.ActivationFunctionType.Sqrt, bias=eps_sb[:])
            nc.vector.reciprocal(rsl[:], rsl[:])
            rsl_bc = tp_.tile([128, 1], f32, tag="rslbc")
            nc.gpsimd.partition_broadcast(rsl_bc[:], rsl[:])
            hl = tp_.tile([128, KC, 1], bf, tag="hlast")
            nc.vector.tensor_mul(
                hl[:], h[:, :, S - 1:S],
                rsl_bc[:, :, None].broadcast_to([128, 1, 1]).broadcast_to([128, KC, 1]),
            )
            pso = ps.tile([128, TT], f32, tag="psum")
            for k in range(KC):
                nc.tensor.matmul(pso[0:OUT, 0:1], wreg_sb[:, k, :], hl[:, k, :],
                                 start=(k == 0), stop=(k == KC - 1))
            ot = tp_.tile([OUT, 1], f32, tag="outt")
            nc.vector.tensor_add(ot[:], pso[0:OUT, 0:1], breg_sb[:])
            nc.sync.dma_start(out_dram[:], ot[:])

    nc.finalize()
    return nc


_CACHED = {}


def _get_program():
    if "nc" not in _CACHED:
        _CACHED["nc"] = build_program()
    return _CACHED["nc"]


def _host_prepare(inputs):
    """Fold norms/scales into weights, gather embeddings, build the 8
    per-core input maps."""
    ids = np.asarray(inputs["input_ids"]).astype(np.int64)        # [B,S]
    amask = np.asarray(inputs["attention_mask"]).astype(np.int64)  # [B,S]
    embed = np.asarray(inputs["embed"], FP32)

    inv_sqrt_hd = FP32(1.0 / np.sqrt(HD))

    # rope tables (half: both halves identical)
    inv = 1.0 / (10000.0 ** (np.arange(0, HD, 2, dtype=np.float64) / HD))
    ang = (np.arange(S, dtype=np.float64)[:, None] * inv[None, :])  # [S,64]
    cos64 = np.cos(ang).T.astype(BF16)  # [64,S]
    sin64 = np.sin(ang).T.astype(BF16)
    cosT = np.concatenate([cos64, cos64], axis=0).copy()  # [128,S]
    sinT = np.concatenate([sin64, sin64], axis=0).copy()

    # causal strip [128, 896]: strip[p,u] = 1 if (u-384) >= p else 0
    u = np.arange(896)[None, :]
    p = np.arange(128)[:, None]
    mstrip = ((u - 384) >= p).astype(BF16)

    per_core = []
    common = {}

    def fold(l):
        g1 = np.asarray(inputs["norm1"], FP32)[l][:, None]
        g2 = np.asarray(inputs["norm2"], FP32)[l][:, None]
        wq = np.asarray(inputs["Wq"], FP32)[l] * g1 * inv_sqrt_hd
        wk = np.asarray(inputs["Wk"], FP32)[l] * g1
        wv = np.asarray(inputs["Wv"], FP32)[l] * g1
        aq = np.asarray(inputs["Aq"], FP32)[l] * g1
        av = np.asarray(inputs["Av"], FP32)[l] * g1
        bq = np.asarray(inputs["Bq"], FP32)[l] * (SCALING * inv_sqrt_hd)
        bv = np.asarray(inputs["Bv"], FP32)[l] * SCALING
        wo = np.asarray(inputs["Wo"], FP32)[l]
        wg = np.asarray(inputs["Wgate"], FP32)[l] * g2
        wu = np.asarray(inputs["Wup"], FP32)[l] * g2
        wd = np.asarray(inputs["Wdown"], FP32)[l]
        return wq, wk, wv, aq, av, bq, bv, wo, wg, wu, wd

    folded = [fold(l) for l in range(L)]
    wregf = (np.asarray(inputs["Wreg"], FP32) * np.asarray(inputs["norm_f"], FP32)[:, None])
    common["wreg"] = wregf.reshape(KC, 128, OUT).astype(BF16)
    common["breg"] = np.asarray(inputs["breg"], FP32).reshape(OUT, 1)
    common["cosT"] = cosT
    common["sinT"] = sinT
    common["mstrip"] = mstrip

    in_maps = []
    for c in range(N_CORES):
        b = c // TP      # batch index (DP group)
        r = c % TP       # TP rank within group
        m = dict(common)
        # embedding gather, transposed, bf16: [D,S] -> [16,128,S] -> [128,16,S]
        xt = embed[ids[b]].T.reshape(KC, 128, S).transpose(1, 0, 2)
        m["xT"] = np.ascontiguousarray(xt).astype(BF16)
        # attention_mask bias [128, TC]: col j, part p -> key token 128j+p
        mb = np.where(amask[b] > 0, FP32(0), FP32(-1e9)).reshape(TC, 128).T
        m["maskbias"] = np.ascontiguousarray(mb)
        for l in range(L):
            wq, wk, wv, aq, av, bq, bv, wo, wg, wu, wd = folded[l]
            dsl = slice(r * DL, (r + 1) * DL)
            fsl = slice(r * FL, (r + 1) * FL)
            m[f"wq{l}"] = np.ascontiguousarray(wq[:, dsl].reshape(KC, 128, DL)).astype(BF16)
            m[f"wk{l}"] = np.ascontiguousarray(wk[:, dsl].reshape(KC, 128, DL)).astype(BF16)
            m[f"wv{l}"] = np.ascontiguousarray(wv[:, dsl].reshape(KC, 128, DL)).astype(BF16)
            m[f"aq{l}"] = np.ascontiguousarray(aq.reshape(KC, 128, R)).astype(BF16)
            m[f"av{l}"] = np.ascontiguousarray(av.reshape(KC, 128, R)).astype(BF16)
            m[f"bq{l}"] = np.ascontiguousarray(bq[:, dsl]).astype(BF16)
            m[f"bv{l}"] = np.ascontiguousarray(bv[:, dsl]).astype(BF16)
            m[f"wo{l}"] = np.ascontiguousarray(wo[dsl].reshape(NH, 128, D)).astype(BF16)
            # wg/wu: [D, FL] -> [FC, KC, 128, 128] (fc, kchunk, part, col)
            wgl = wg[:, fsl].reshape(KC, 128, FC, 128).transpose(2, 0, 1, 3)
            wul = wu[:, fsl].reshape(KC, 128, FC, 128).transpose(2, 0, 1, 3)
            m[f"wg{l}"] = np.ascontiguousarray(wgl).astype(BF16)
            m[f"wu{l}"] = np.ascontiguousarray(wul).astype(BF16)
            m[f"wd{l}"] = np.ascontiguousarray(wd[fsl].reshape(FC, 128, D)).astype(BF16)
        in_maps.append(m)
    return in_maps


def run_on_device(in_maps, trace=False):
    nc = _get_program()
    return bass_utils.run_bass_kernel_spmd(
        nc, in_maps, core_ids=list(range(N_CORES)), trace=trace,
    )


def kernel(**inputs):
    in_maps = _host_prepare(inputs)
    res = run_on_device(in_maps, trace=False)
    out = np.stack([
        res.results[0]["out"].reshape(OUT),
        res.results[TP]["out"].reshape(OUT),
    ]).astype(FP32)
    return out
